# revision 10
# baseline (speedup 1.0000x reference)
"""Trainium2 Bass kernel for nn_CausalGraphLearner.

Computes, for each batch b and slot pair (i, j):
    x    = cat([s_i, s_j, s_i - s_j, s_i * s_j])            # [4D]
    h1   = x @ W1 + b1                                      # [H]
    h    = gelu(LayerNorm(h1))                              # exact gelu
    h2   = gelu(h @ W2 + b2)
    out  = sigmoid(h2 @ W3 + b3)                            # scalar
Output: [B, N, N] with B=8, N=256, D=64, H=256.

Strategy: data-parallel over B across the 8 NeuronCores (1 batch per core).
The first Linear factors as
    h1 = s_j@(Wb-Wc) + (s_i*s_j)@Wd + [s_i@(Wa+Wc) + b1]
so per row-index i we run one K=128 matmul (lhsT = [slotsT; s_i*slotsT]) plus
a rank-1 accumulate for the i-dependent row broadcast.  The final Linear
(h2 @ W3) runs as one-column matmuls with z2g as the stationary operand,
producing the output transposed; the host transposes it back.
"""

import os
import sys

sys.path.insert(0, "/opt/trn_rl_repo")

import numpy as np
import ml_dtypes

import concourse.bass as bass
import concourse.tile as tile
from concourse import bacc, mybir
from concourse.bass_utils import run_bass_kernel_spmd

B, N, D = 8, 256, 64
H = 256
K2 = H // 2  # 128
LN_EPS = 1e-5
NCORES = 8

F32 = mybir.dt.float32
BF16 = mybir.dt.bfloat16
U32 = mybir.dt.uint32
I32 = mybir.dt.int32
AF = mybir.ActivationFunctionType
ALU = mybir.AluOpType

MAGIC = 0x5F3759DF  # fast inverse-sqrt seed

_prog_cache = {}


def _build_program(b3: float) -> bass.Bass:
    nc = bacc.Bacc(
        "TRN2", target_bir_lowering=False, debug=False, num_devices=NCORES
    )

    slotst_f = nc.declare_dram_parameter("slotst_f", [D, N], F32, False)
    slotst_b = nc.declare_dram_parameter("slotst_b", [D, N], BF16, False)
    wbwd_d = nc.declare_dram_parameter("wbwd", [2 * D, H], BF16, False)
    utab_d = nc.declare_dram_parameter("utab", [N, H], BF16, False)
    w2_d = nc.declare_dram_parameter("w2", [128, 2, K2], BF16, False)
    w3_d = nc.declare_dram_parameter("w3", [K2, 1], BF16, False)
    b2_d = nc.declare_dram_parameter("b2", [K2, 1], F32, False)
    # outT[j, i] = sigmoid-logit for pair (i, j); host transposes back.
    outT_d = nc.declare_dram_parameter("outT", [2, 128, 2, 128], F32, True)
    acts_d = nc.dram_tensor("actscratch", [2, 8, N, H], BF16)

    NH = 5   # h1 psum ring depth (banks)
    BATCH = 4  # stats-merge batch (i's per merge)

    with tile.TileContext(nc) as tc:
        with (
            tc.tile_pool(name="const", bufs=1) as cpool,
            tc.tile_pool(name="work", bufs=1) as wpool,
            tc.tile_pool(name="tmp", bufs=2) as spool,
            tc.tile_pool(name="psum", bufs=1, space="PSUM") as ppool,
        ):
            # ---- constants / parameters in SBUF ----
            combs = [cpool.tile([128, N], BF16, name=f"comb{k}", tag=f"comb{k}") for k in range(4)]
            slotshi = cpool.tile([128, N], F32, name="slotshi", tag="slotshi")
            slotsbh = cpool.tile([128, N], BF16, name="slotsbh", tag="slotsbh")
            wbwd = cpool.tile([128, H], BF16, name="wbwd", tag="wbwd")
            ustage = [cpool.tile([1, BATCH, H], BF16, name=f"ustage{k}", tag=f"ustage{k}") for k in range(2)]
            w2t = cpool.tile([128, 2, K2], BF16, name="w2", tag="w2")
            w3t = cpool.tile([K2, 1], BF16, name="w3", tag="w3")
            b2t = cpool.tile([K2, 1], F32, name="b2", tag="b2")
            ones = cpool.tile([1, 128], BF16, name="ones", tag="ones")
            b3t = cpool.tile([128, 1], F32, name="b3t", tag="b3t")

            for k in range(4):
                nc.sync.dma_start(combs[k][0:D, :], slotst_b[:, :])
            nc.sync.dma_start(slotshi[D:128, :], slotst_f[:, :])
            nc.sync.dma_start(slotsbh[D:128, :], slotst_b[:, :])
            nc.sync.dma_start(wbwd[:], wbwd_d[:, :])
            nc.sync.dma_start(w2t[:], w2_d[:, :, :])
            nc.sync.dma_start(w3t[:], w3_d[:, :])
            nc.sync.dma_start(b2t[:], b2_d[:, :])
            nc.vector.memset(ones[:], 1.0)
            nc.vector.memset(b3t[:], float(b3) * 0.5)

            # ---- PSUM layout: 5 + 2 + 1 = 8 banks exactly ----
            h1r = [ppool.tile([128, 2, H], F32, name=f"h1_{m}", tag=f"h1_{m}") for m in range(NH)]
            z2p = [ppool.tile([128, 2, N], F32, name=f"z2p{m}", tag=f"z2p{m}") for m in range(2)]
            lT = ppool.tile([128, 2, 2, 128], F32, name="lT", tag="lT")

            # ---- SBUF work rings ----
            actr = [wpool.tile([128, BATCH, 2, H], BF16, name=f"act{m}", tag=f"act{m}") for m in range(3)]
            actT8 = [wpool.tile([128, 2, 8, N], BF16, name=f"actT8_{m}", tag=f"actT8_{m}") for m in range(2)]
            z2g = [wpool.tile([128, 2, N], BF16, name=f"z2g{m}", tag=f"z2g{m}") for m in range(2)]
            stats = [wpool.tile([128, BATCH, 2, 6], F32, name=f"stats{m}", tag=f"stats{m}") for m in range(3)]
            rstd = [wpool.tile([128, BATCH, 2], F32, name=f"rstd{m}", tag=f"rstd{m}") for m in range(3)]
            nbias = [wpool.tile([128, BATCH, 2], F32, name=f"nbias{m}", tag=f"nbias{m}") for m in range(3)]
            sig = [wpool.tile([128, 2, 128], F32, name=f"sig{m}", tag=f"sig{m}") for m in range(2)]
            outsb = [wpool.tile([128, 2, 128], F32, name=f"outsb{m}", tag=f"outsb{m}") for m in range(2)]

            def merge_and_rsqrt(k: int):
                """From bn_stats of batch k produce rstd = 1/sqrt(var+eps) and
                nbias = -mean*rstd for the 4 i's of the batch."""
                w = k % 3
                st = stats[w]
                mE = st[:, :, :, 1]
                M2E = st[:, :, :, 2]
                mO = st[:, :, :, 4]
                M2O = st[:, :, :, 5]
                shp = [128, BATCH, 2]

                tB = spool.tile(shp, F32, tag="tB")
                tS = spool.tile(shp, F32, tag="tS")
                tBB = spool.tile(shp, F32, tag="tBB")
                tv1 = spool.tile(shp, F32, tag="tv1")
                tvar = spool.tile(shp, F32, tag="tvar")
                nc.vector.tensor_tensor(tB[:], mE, mO, ALU.subtract)
                nc.vector.tensor_tensor(tS[:], M2E, M2O, ALU.add)
                nc.vector.tensor_tensor(tBB[:], tB[:], tB[:], ALU.mult)
                nc.vector.tensor_scalar(tv1[:], tS[:], 1.0 / H, None, ALU.mult)
                # var = S/H + (B/2)^2 + eps
                nc.vector.tensor_scalar(tBB[:], tBB[:], 0.25, LN_EPS, ALU.mult, ALU.add)
                nc.vector.tensor_tensor(tvar[:], tv1[:], tBB[:], ALU.add)

                # Newton rsqrt with bit-trick seed: r0_bits = MAGIC - (bits>>1)
                ti = spool.tile(shp, I32, tag="ti")
                nc.vector.tensor_scalar(
                    ti[:], tvar[:].bitcast(I32), 1, None, ALU.logical_shift_right
                )
                nc.vector.tensor_scalar(ti[:], ti[:], -1, MAGIC, ALU.mult, ALU.add)
                r = ti[:].bitcast(F32)
                ta = spool.tile(shp, F32, tag="ta")
                tb2 = spool.tile(shp, F32, tag="tb2")
                for it in range(1):
                    dest = rstd[w]
                    nc.vector.tensor_tensor(ta[:], r, r, ALU.mult)
                    nc.vector.tensor_tensor(ta[:], ta[:], tvar[:], ALU.mult)
                    nc.vector.tensor_scalar(tb2[:], ta[:], -0.5, 1.5, ALU.mult, ALU.add)
                    nc.vector.tensor_tensor(dest[:], r, tb2[:], ALU.mult)
                    r = dest[:]
                # nbias = -mean * rstd ; mean = (mE+mO)/2
                tA = spool.tile(shp, F32, tag="tA")
                nc.vector.tensor_tensor(tA[:], mE, mO, ALU.add)
                nc.vector.tensor_tensor(tA[:], tA[:], rstd[w][:], ALU.mult)
                nc.vector.tensor_scalar(nbias[w][:], tA[:], -0.5, None, ALU.mult)

            # ---- main loop, software-pipelined in batches of BATCH ----
            NB = N // BATCH

            def stage_u(k: int):
                """Prefetch batch k's u-rows into the ustage ring."""
                if k >= NB:
                    return
                nc.gpsimd.dma_start(
                    ustage[k % 2][0:1, :, :],
                    utab_d[BATCH * k : BATCH * (k + 1), :].rearrange(
                        "(o a) b -> o a b", o=1
                    ),
                )

            def phase_a(k: int):
                """mT, mm1, bn_stats for the 4 i's of batch k."""
                stage_u(k + 1)
                # all mT multiplies first so the PE never waits on the DVE
                for i in range(BATCH * k, BATCH * (k + 1)):
                    mc = i % 4
                    # mT = s_i * slotsT on partitions 64..127 (bf16 in, DVE)
                    nc.vector.tensor_scalar(
                        combs[mc][D:128, :],
                        slotsbh[D:128, :],
                        slotshi[D:128, i : i + 1],
                        None,
                        ALU.mult,
                    )
                for i in range(BATCH * k, BATCH * (k + 1)):
                    m5 = i % NH
                    mc = i % 4
                    bi = i % BATCH

                    # h1 = comb.T @ [WB; Wd]  (+ rank-1 of (u_i + b1))
                    h1 = h1r[m5]
                    nc.tensor.matmul(
                        h1[:, 0, :], combs[mc][:, 0:128], wbwd[:], start=True, stop=False
                    )
                    nc.tensor.matmul(
                        h1[:, 1, :], combs[mc][:, 128:256], wbwd[:], start=False, stop=False
                    )
                    urow = ustage[(i // BATCH) % 2][0:1, bi, :]
                    nc.tensor.matmul(h1[:, 0, :], ones[:], urow, start=False, stop=False)
                    nc.tensor.matmul(h1[:, 1, :], ones[:], urow, start=False, stop=True)
                for i in range(BATCH * k, BATCH * (k + 1)):
                    m5 = i % NH
                    w = k % 3
                    bi = i % BATCH
                    h1 = h1r[m5]
                    # LayerNorm stats (per j-chunk; grouped bn_stats would be
                    # flattened by AP opt and mix the chunks)
                    for c in range(2):
                        nc.vector.bn_stats(stats[w][:, bi, c, :], h1[:, c, :])

            def phase_b(k: int):
                """gelu1 + act scratch write for the 4 i's of batch k."""
                m3a = k % 3
                for i in range(BATCH * k, BATCH * (k + 1)):
                    m5 = i % NH
                    w = k % 3
                    bi = i % BATCH
                    h1 = h1r[m5]

                    # act = gelu((h1 - mean) * rstd)  [per-chunk scale/bias]
                    for c in range(2):
                        nc.scalar.activation(
                            actr[m3a][:, bi, c, :],
                            h1[:, c, :],
                            AF.Gelu,
                            bias=nbias[w][:, bi, c : c + 1],
                            scale=rstd[w][:, bi, c : c + 1],
                        )
                # stage the whole batch's act to DRAM scratch (SWDGE, one DMA)
                nc.gpsimd.dma_start(
                    acts_d[(k // 2) % 2, BATCH * (k % 2) : BATCH * (k % 2 + 1), :, :]
                    .rearrange("i (c p) h -> p i c h", c=2),
                    actr[m3a][:, :, :, :],
                )

            def phase_c(m: int):
                """block of 8 i's: batched DMA transposes, then mm2/gelu2/mm3."""
                m2 = m % 2
                # actT[h', d, ip, j] = act[i=8m+ip][j, 128d + h']
                for d in range(2):
                    nc.sync.dma_start_transpose(
                        actT8[m2][:, d, :, :].rearrange("p a b -> p (a b)"),
                        acts_d[m2, :, :, 128 * d : 128 * (d + 1)].rearrange(
                            "a b c -> (a b) c"
                        ),
                    )
                for i in range(8 * m, 8 * (m + 1)):
                    ip = i % 8
                    blk = i // 128
                    pair = i // 2

                    # z2T[k, j] = W2.T-chunks @ actT  (double-buffered per pair)
                    zp = z2p[pair % 2]
                    for hc in range(2):
                        nc.tensor.matmul(
                            zp[:, i % 2, :],
                            w2t[:, hc, :],
                            actT8[m2][:, hc, ip, :],
                            start=(i % 2 == 0 and hc == 0),
                            stop=(i % 2 == 1 and hc == 1),
                        )

                    if i % 2 == 1:
                        # gelu2 batched over the pair; b2 is a per-partition bias
                        nc.scalar.activation(
                            z2g[pair % 2][:, :, :],
                            zp[:, :, :],
                            AF.Gelu,
                            bias=b2t[:, 0:1],
                            scale=1.0,
                        )
                        # logits for the pair: one-column matmuls with z2g as
                        # the stationary operand; row lands transposed in lT
                        for par in range(2):
                            ii = i - 1 + par
                            it = ii % 128
                            for c in range(2):
                                nc.tensor.matmul(
                                    lT[:, blk % 2, c, it : it + 1],
                                    z2g[pair % 2][:, par, 128 * c : 128 * (c + 1)],
                                    w3t[:],
                                    start=True,
                                    stop=True,
                                )

                    if i % 128 == 127:
                        # sigmoid(x + b3) = 0.5 + 0.5*tanh((x + b3)/2); tanh is
                        # in the gelu table set, so no ACT table reload.
                        nc.scalar.activation(
                            sig[blk % 2][:],
                            lT[:, blk % 2, :, :],
                            AF.Tanh,
                            bias=b3t[:, 0:1],
                            scale=0.5,
                        )
                        nc.vector.tensor_scalar(
                            outsb[blk % 2][:], sig[blk % 2][:], 0.5, 0.5, ALU.mult, ALU.add
                        )
                        nc.gpsimd.dma_start(
                            outT_d[:, :, blk, :].rearrange("c p t -> p c t"),
                            outsb[blk % 2][:],
                        )

            stage_u(0)
            for k in range(NB):
                phase_a(k)
                merge_and_rsqrt(k)
                # phase_c for a block whose acts are a full batch old, so its
                # transposes/matmuls never stall on the just-written scratch
                if k % 2 == 0 and k >= 2:
                    phase_c(k // 2 - 1)
                phase_b(k)
            phase_c(NB // 2 - 1)

    nc.finalize()
    return nc


def _np_reference(slots, W1, b1, ln_g, ln_b, W2, b2, W3, b3):
    """Exact fallback (only used if ln_g/ln_b are not identity)."""
    import jax
    import jax.numpy as jnp

    si = slots[:, :, None, :]
    sj = slots[:, None, :, :]
    d = slots.shape[-1]
    Wa, Wb, Wc, Wd = W1[:d], W1[d : 2 * d], W1[2 * d : 3 * d], W1[3 * d :]
    h = (
        jnp.einsum("bnd,dh->bnh", slots, Wa + Wc)[:, :, None, :]
        + jnp.einsum("bnd,dh->bnh", slots, Wb - Wc)[:, None, :, :]
        + jnp.einsum("bxyd,dh->bxyh", si * sj, Wd)
        + b1
    )
    mu = jnp.mean(h, axis=-1, keepdims=True)
    var = jnp.mean(jnp.square(h - mu), axis=-1, keepdims=True)
    h = (h - mu) * jax.lax.rsqrt(var + LN_EPS) * ln_g + ln_b
    h = jax.nn.gelu(h, approximate=False)
    h = jax.nn.gelu(jnp.einsum("bxyh,hk->bxyk", h, W2) + b2, approximate=False)
    logits = (jnp.einsum("bxyk,ko->bxyo", h, W3) + b3)[..., 0]
    return np.asarray(jax.nn.sigmoid(logits), dtype=np.float32)


def kernel(slots, W1, b1, ln_g, ln_b, W2, b2, W3, b3):
    slots = np.asarray(slots, dtype=np.float32)
    W1 = np.asarray(W1, dtype=np.float32)
    b1 = np.asarray(b1, dtype=np.float32)
    ln_g = np.asarray(ln_g, dtype=np.float32)
    ln_b = np.asarray(ln_b, dtype=np.float32)
    W2 = np.asarray(W2, dtype=np.float32)
    b2 = np.asarray(b2, dtype=np.float32)
    W3 = np.asarray(W3, dtype=np.float32)
    b3 = np.asarray(b3, dtype=np.float32)

    if not (np.allclose(ln_g, 1.0) and np.allclose(ln_b, 0.0)):
        return _np_reference(slots, W1, b1, ln_g, ln_b, W2, b2, W3, b3)

    Wa, Wb, Wc, Wd = W1[:D], W1[D : 2 * D], W1[2 * D : 3 * D], W1[3 * D :]
    WA = Wa + Wc  # [64, 256]
    wbwd = np.concatenate([Wb - Wc, Wd], axis=0)  # [128, 256]
    b3f = float(b3.reshape(-1)[0])

    key = b3f
    if key not in _prog_cache:
        _prog_cache[key] = _build_program(b3f)
    nc = _prog_cache[key]

    bf = ml_dtypes.bfloat16
    wbwd_b = wbwd.astype(bf)
    w2s = np.ascontiguousarray(
        np.transpose(W2.reshape(2, 128, K2), (1, 0, 2))
    ).astype(bf)  # [128h', 2hc, 128k]
    w3s = W3.reshape(K2, 1).astype(bf)
    b2s = b2.reshape(K2, 1).astype(np.float32)

    in_maps = []
    for b in range(B):
        sT = np.ascontiguousarray(slots[b].T)  # [64, 256] f32
        utab_s = (slots[b] @ WA + b1).astype(bf)  # [256, 256]
        in_maps.append(
            {
                "slotst_f": sT,
                "slotst_b": sT.astype(bf),
                "wbwd": wbwd_b,
                "utab": utab_s,
                "w2": w2s,
                "w3": w3s,
                "b2": b2s,
            }
        )

    trace = os.environ.get("KERNEL_TRACE", "0") == "1"
    try:
        res = run_bass_kernel_spmd(nc, in_maps, list(range(NCORES)), trace=trace)
    except ModuleNotFoundError:
        res = run_bass_kernel_spmd(nc, in_maps, list(range(NCORES)), trace=False)
    kernel.last_result = res
    if trace and res.exec_time_ns is not None:
        print(f"HW exec time: {res.exec_time_ns} ns")
        kernel.last_exec_time_ns = res.exec_time_ns
    out = np.stack(
        [res.results[b]["outT"].reshape(N, N).T for b in range(B)], axis=0
    )
    return np.ascontiguousarray(out).astype(np.float32)


kernel.last_exec_time_ns = None
kernel.last_result = None


# revision 26
# speedup vs baseline: 1.0966x; 1.0966x over previous
"""Trainium2 Bass kernel for nn_CausalGraphLearner.

Computes, for each batch b and slot pair (i, j):
    x    = cat([s_i, s_j, s_i - s_j, s_i * s_j])            # [4D]
    h1   = x @ W1 + b1                                      # [H]
    h    = gelu(LayerNorm(h1))                              # exact gelu
    h2   = gelu(h @ W2 + b2)
    out  = sigmoid(h2 @ W3 + b3)                            # scalar
Output: [B, N, N] with B=8, N=256, D=64, H=256.

Strategy: data-parallel over B across the 8 NeuronCores (1 batch per core).
The first Linear factors as
    h1 = s_j@(Wb-Wc) + (s_i*s_j)@Wd + [s_i@(Wa+Wc) + b1]
so per row-index i we run one K=128 matmul (lhsT = [slotsT; s_i*slotsT]) plus
a rank-1 accumulate for the i-dependent row broadcast.  The final Linear
(h2 @ W3) runs as one-column matmuls with z2g as the stationary operand,
producing the output transposed; the host transposes it back.
"""

import os
import sys

sys.path.insert(0, "/opt/trn_rl_repo")

import numpy as np
import ml_dtypes

import concourse.bass as bass
import concourse.tile as tile
from concourse import bacc, mybir
from concourse.bass_utils import run_bass_kernel_spmd

B, N, D = 8, 256, 64
H = 256
K2 = H // 2  # 128
LN_EPS = 1e-5
NCORES = 8

F32 = mybir.dt.float32
BF16 = mybir.dt.bfloat16
U32 = mybir.dt.uint32
I32 = mybir.dt.int32
AF = mybir.ActivationFunctionType
ALU = mybir.AluOpType

MAGIC = 0x5F3759DF  # fast inverse-sqrt seed

_prog_cache = {}


def _build_program(b3: float) -> bass.Bass:
    nc = bacc.Bacc(
        "TRN2", target_bir_lowering=False, debug=False, num_devices=NCORES
    )

    slotst_f = nc.declare_dram_parameter("slotst_f", [D, N], F32, False)
    slotst_b = nc.declare_dram_parameter("slotst_b", [D, N], BF16, False)
    wbwd_d = nc.declare_dram_parameter("wbwd", [2 * D, H], BF16, False)
    utab_d = nc.declare_dram_parameter("utab", [N, H], BF16, False)
    w2_d = nc.declare_dram_parameter("w2", [128, 2, K2], BF16, False)
    w3_d = nc.declare_dram_parameter("w3", [K2, 1], BF16, False)
    b2_d = nc.declare_dram_parameter("b2", [K2, 1], F32, False)
    # outT[j, i] = sigmoid-logit for pair (i, j); host transposes back.
    outT_d = nc.declare_dram_parameter("outT", [2, 128, 2, 128], F32, True)
    acts_d = nc.dram_tensor("actscratch", [3, 8, N, H], BF16)

    NH = 5   # h1 psum ring depth (banks)
    BATCH = 4  # stats-merge batch (i's per merge)

    with tile.TileContext(nc) as tc:
        with (
            tc.tile_pool(name="const", bufs=1) as cpool,
            tc.tile_pool(name="work", bufs=1) as wpool,
            tc.tile_pool(name="tmp", bufs=2) as spool,
            tc.tile_pool(name="psum", bufs=1, space="PSUM") as ppool,
        ):
            # ---- constants / parameters in SBUF ----
            combs = [cpool.tile([128, N], BF16, name=f"comb{k}", tag=f"comb{k}") for k in range(8)]
            slotshi = cpool.tile([128, N], F32, name="slotshi", tag="slotshi")
            slotsbh = cpool.tile([128, N], BF16, name="slotsbh", tag="slotsbh")
            wbwd = cpool.tile([128, H], BF16, name="wbwd", tag="wbwd")
            ustage = [cpool.tile([1, BATCH, H], BF16, name=f"ustage{k}", tag=f"ustage{k}") for k in range(3)]
            w2t = cpool.tile([128, 2, K2], BF16, name="w2", tag="w2")
            w3t = cpool.tile([K2, 1], BF16, name="w3", tag="w3")
            b2t = cpool.tile([K2, 1], F32, name="b2", tag="b2")
            ones = cpool.tile([1, 128], BF16, name="ones", tag="ones")
            b3t = cpool.tile([128, 1], F32, name="b3t", tag="b3t")

            for k in range(8):
                nc.sync.dma_start(combs[k][0:D, :], slotst_b[:, :])
            nc.sync.dma_start(slotshi[D:128, :], slotst_f[:, :])
            nc.sync.dma_start(slotsbh[D:128, :], slotst_b[:, :])
            nc.sync.dma_start(wbwd[:], wbwd_d[:, :])
            nc.sync.dma_start(w2t[:], w2_d[:, :, :])
            nc.sync.dma_start(w3t[:], w3_d[:, :])
            nc.sync.dma_start(b2t[:], b2_d[:, :])
            nc.vector.memset(ones[:], 1.0)
            nc.vector.memset(b3t[:], float(b3) * 0.5)

            # ---- PSUM layout: 5 + 2 + 1 = 8 banks exactly ----
            h1r = [ppool.tile([128, 2, H], F32, name=f"h1_{m}", tag=f"h1_{m}") for m in range(NH)]
            z2p = [ppool.tile([128, 2, N], F32, name=f"z2p{m}", tag=f"z2p{m}") for m in range(2)]
            lT = ppool.tile([128, 2, 2, 128], F32, name="lT", tag="lT")

            # ---- SBUF work rings ----
            actr = [wpool.tile([128, BATCH, 2, H], BF16, name=f"act{m}", tag=f"act{m}") for m in range(3)]
            actT8 = [wpool.tile([128, 2, 8, N], BF16, name=f"actT8_{m}", tag=f"actT8_{m}") for m in range(2)]
            z2g = [wpool.tile([128, 2, N], BF16, name=f"z2g{m}", tag=f"z2g{m}") for m in range(4)]
            stats = [wpool.tile([128, BATCH, 2, 6], F32, name=f"stats{m}", tag=f"stats{m}") for m in range(3)]
            mvar = [wpool.tile([128, BATCH, 2, 2], F32, name=f"mvar{m}", tag=f"mvar{m}") for m in range(3)]
            rstd = [wpool.tile([128, BATCH, 2], F32, name=f"rstd{m}", tag=f"rstd{m}") for m in range(3)]
            nbias = [wpool.tile([128, BATCH, 2], F32, name=f"nbias{m}", tag=f"nbias{m}") for m in range(3)]
            sig = [wpool.tile([128, 2, 128], F32, name=f"sig{m}", tag=f"sig{m}") for m in range(2)]
            outsb = [wpool.tile([128, 2, 128], F32, name=f"outsb{m}", tag=f"outsb{m}") for m in range(2)]

            def merge_and_rsqrt(k: int):
                """Merge the per-window bn_stats of batch k into mean/var,
                then a short Newton-rsqrt chain on the batch."""
                w = k % 3
                st = stats[w]
                mE = st[:, :, :, 1]
                M2E = st[:, :, :, 2]
                mO = st[:, :, :, 4]
                M2O = st[:, :, :, 5]
                shp = [128, BATCH, 2]

                tB = spool.tile(shp, F32, tag="tB")
                tS = spool.tile(shp, F32, tag="tS")
                tBB = spool.tile(shp, F32, tag="tBB")
                tv1 = spool.tile(shp, F32, tag="tv1")
                tvar = spool.tile(shp, F32, tag="tvar")
                mean = spool.tile(shp, F32, tag="mean")
                nc.vector.tensor_tensor(tB[:], mE, mO, ALU.subtract)
                nc.vector.tensor_tensor(tS[:], M2E, M2O, ALU.add)
                nc.vector.tensor_tensor(tBB[:], tB[:], tB[:], ALU.mult)
                nc.vector.tensor_scalar(tv1[:], tS[:], 1.0 / H, None, ALU.mult)
                # var = S/H + (B/2)^2 + eps
                nc.vector.tensor_scalar(tBB[:], tBB[:], 0.25, LN_EPS, ALU.mult, ALU.add)
                nc.vector.tensor_tensor(tvar[:], tv1[:], tBB[:], ALU.add)
                nc.vector.tensor_tensor(mean[:], mE, mO, ALU.add)
                nc.vector.tensor_scalar(mean[:], mean[:], 0.5, None, ALU.mult)
                # Newton rsqrt with bit-trick seed: r0_bits = MAGIC - (bits>>1)
                ti = spool.tile(shp, I32, tag="ti")
                nc.vector.tensor_scalar(
                    ti[:], tvar[:].bitcast(I32), 1, None, ALU.logical_shift_right
                )
                nc.vector.tensor_scalar(ti[:], ti[:], -1, MAGIC, ALU.mult, ALU.add)
                r = ti[:].bitcast(F32)
                ta = spool.tile(shp, F32, tag="ta")
                tb2 = spool.tile(shp, F32, tag="tb2")
                dest = rstd[w]
                nc.vector.tensor_tensor(ta[:], r, r, ALU.mult)
                nc.vector.tensor_tensor(ta[:], ta[:], tvar[:], ALU.mult)
                nc.vector.tensor_scalar(tb2[:], ta[:], -0.5, 1.5, ALU.mult, ALU.add)
                nc.vector.tensor_tensor(dest[:], r, tb2[:], ALU.mult)
                # nbias = -mean * rstd
                tA = spool.tile(shp, F32, tag="tA")
                nc.vector.tensor_tensor(tA[:], mean[:], dest[:], ALU.mult)
                nc.vector.tensor_scalar(nbias[w][:], tA[:], -1.0, None, ALU.mult)

            # ---- main loop, software-pipelined in batches of BATCH ----
            NB = N // BATCH

            def stage_u(k: int):
                """Prefetch batch k's u-rows into the ustage ring."""
                if k >= NB:
                    return
                nc.gpsimd.dma_start(
                    ustage[k % 3][0:1, :, :],
                    utab_d[BATCH * k : BATCH * (k + 1), :].rearrange(
                        "(o a) b -> o a b", o=1
                    ),
                )

            def phase_mt(k: int):
                """mT multiplies for batch k (emitted one batch early so the
                DVE queue never blocks the next batch's matmuls)."""
                if k >= NB:
                    return
                for i in range(BATCH * k, BATCH * (k + 1)):
                    mc = i % 8
                    # mT = s_i * slotsT on partitions 64..127 (bf16 in, DVE)
                    nc.vector.tensor_scalar(
                        combs[mc][D:128, :],
                        slotsbh[D:128, :],
                        slotshi[D:128, i : i + 1],
                        None,
                        ALU.mult,
                    )

            def phase_a(k: int):
                """mm1 + bn_stats for the 4 i's of batch k."""
                stage_u(k + 2)
                phase_mt(k + 1)
                for i in range(BATCH * k, BATCH * (k + 1)):
                    m5 = i % NH
                    mc = i % 8
                    bi = i % BATCH

                    # h1 = comb.T @ [WB; Wd]  (+ rank-1 of (u_i + b1))
                    h1 = h1r[m5]
                    nc.tensor.matmul(
                        h1[:, 0, :], combs[mc][:, 0:128], wbwd[:], start=True, stop=False
                    )
                    nc.tensor.matmul(
                        h1[:, 1, :], combs[mc][:, 128:256], wbwd[:], start=False, stop=False
                    )
                    urow = ustage[(i // BATCH) % 3][0:1, bi, :]
                    nc.tensor.matmul(h1[:, 0, :], ones[:], urow, start=False, stop=False)
                    nc.tensor.matmul(h1[:, 1, :], ones[:], urow, start=False, stop=True)
                for i in range(BATCH * k, BATCH * (k + 1)):
                    m5 = i % NH
                    w = k % 3
                    bi = i % BATCH
                    h1 = h1r[m5]
                    # LayerNorm stats (per j-chunk; grouped bn_stats would be
                    # flattened by AP opt and mix the chunks)
                    for c in range(2):
                        nc.vector.bn_stats(stats[w][:, bi, c, :], h1[:, c, :])

            def phase_b(k: int):
                """gelu1 + act scratch write for the 4 i's of batch k."""
                m3a = k % 3
                for i in range(BATCH * k, BATCH * (k + 1)):
                    m5 = i % NH
                    w = k % 3
                    bi = i % BATCH
                    h1 = h1r[m5]

                    # act = gelu((h1 - mean) * rstd)  [per-chunk scale/bias]
                    for c in range(2):
                        nc.scalar.activation(
                            actr[m3a][:, bi, c, :],
                            h1[:, c, :],
                            AF.Gelu,
                            bias=nbias[w][:, bi, c : c + 1],
                            scale=rstd[w][:, bi, c : c + 1],
                        )
                # stage the whole batch's act to DRAM scratch (SWDGE, one DMA)
                nc.gpsimd.dma_start(
                    acts_d[(k // 2) % 3, BATCH * (k % 2) : BATCH * (k % 2 + 1), :, :]
                    .rearrange("i (c p) h -> p i c h", c=2),
                    actr[m3a][:, :, :, :],
                )

            def phase_c_dma(m: int):
                """batched DMA transposes for block m (emitted right after the
                block's last acts write so the transfer overlaps compute)."""
                m2 = m % 2
                # actT[h', d, ip, j] = act[i=8m+ip][j, 128d + h']
                for d in range(2):
                    nc.sync.dma_start_transpose(
                        actT8[m2][:, d, :, :].rearrange("p a b -> p (a b)"),
                        acts_d[m % 3, :, :, 128 * d : 128 * (d + 1)].rearrange(
                            "a b c -> (a b) c"
                        ),
                    )

            def phase_c(m: int):
                """block of 8 i's: mm2/gelu2/mm3 on pre-transposed acts."""
                m2 = m % 2
                for i in range(8 * m, 8 * (m + 1)):
                    ip = i % 8
                    blk = i // 128
                    pair = i // 2

                    # z2T[k, j] = W2.T-chunks @ actT  (double-buffered per pair)
                    zp = z2p[pair % 2]
                    for hc in range(2):
                        nc.tensor.matmul(
                            zp[:, i % 2, :],
                            w2t[:, hc, :],
                            actT8[m2][:, hc, ip, :],
                            start=(i % 2 == 0 and hc == 0),
                            stop=(i % 2 == 1 and hc == 1),
                        )

                    if i % 2 == 1:
                        # gelu2 batched over the pair; b2 is a per-partition bias
                        nc.scalar.activation(
                            z2g[pair % 4][:, :, :],
                            zp[:, :, :],
                            AF.Gelu,
                            bias=b2t[:, 0:1],
                            scale=1.0,
                        )
                        # logits for the pair: one-column matmuls with z2g as
                        # the stationary operand; row lands transposed in lT
                        for par in range(2):
                            ii = i - 1 + par
                            it = ii % 128
                            for c in range(2):
                                nc.tensor.matmul(
                                    lT[:, blk % 2, c, it : it + 1],
                                    z2g[pair % 4][:, par, 128 * c : 128 * (c + 1)],
                                    w3t[:],
                                    start=True,
                                    stop=True,
                                )

                    if i % 128 == 127:
                        # sigmoid(x + b3) = 0.5 + 0.5*tanh((x + b3)/2); tanh is
                        # in the gelu table set, so no ACT table reload.
                        nc.scalar.activation(
                            sig[blk % 2][:],
                            lT[:, blk % 2, :, :],
                            AF.Tanh,
                            bias=b3t[:, 0:1],
                            scale=0.5,
                        )
                        nc.vector.tensor_scalar(
                            outsb[blk % 2][:], sig[blk % 2][:], 0.5, 0.5, ALU.mult, ALU.add
                        )
                        nc.gpsimd.dma_start(
                            outT_d[:, :, blk, :].rearrange("c p t -> p c t"),
                            outsb[blk % 2][:],
                        )

            stage_u(0)
            stage_u(1)
            phase_mt(0)
            for k in range(NB):
                phase_a(k)
                merge_and_rsqrt(k)
                # phase_c compute for a block whose acts are 2+ batches old
                # (transposes were issued at the end of cycle 2m+1)
                if k % 2 == 0 and k >= 4:
                    phase_c(k // 2 - 2)
                phase_b(k)
                if k % 2 == 1:
                    phase_c_dma((k - 1) // 2)
            phase_c(NB // 2 - 2)
            phase_c(NB // 2 - 1)

    nc.finalize()
    return nc


def _np_reference(slots, W1, b1, ln_g, ln_b, W2, b2, W3, b3):
    """Exact fallback (only used if ln_g/ln_b are not identity)."""
    import jax
    import jax.numpy as jnp

    si = slots[:, :, None, :]
    sj = slots[:, None, :, :]
    d = slots.shape[-1]
    Wa, Wb, Wc, Wd = W1[:d], W1[d : 2 * d], W1[2 * d : 3 * d], W1[3 * d :]
    h = (
        jnp.einsum("bnd,dh->bnh", slots, Wa + Wc)[:, :, None, :]
        + jnp.einsum("bnd,dh->bnh", slots, Wb - Wc)[:, None, :, :]
        + jnp.einsum("bxyd,dh->bxyh", si * sj, Wd)
        + b1
    )
    mu = jnp.mean(h, axis=-1, keepdims=True)
    var = jnp.mean(jnp.square(h - mu), axis=-1, keepdims=True)
    h = (h - mu) * jax.lax.rsqrt(var + LN_EPS) * ln_g + ln_b
    h = jax.nn.gelu(h, approximate=False)
    h = jax.nn.gelu(jnp.einsum("bxyh,hk->bxyk", h, W2) + b2, approximate=False)
    logits = (jnp.einsum("bxyk,ko->bxyo", h, W3) + b3)[..., 0]
    return np.asarray(jax.nn.sigmoid(logits), dtype=np.float32)


def kernel(slots, W1, b1, ln_g, ln_b, W2, b2, W3, b3):
    slots = np.asarray(slots, dtype=np.float32)
    W1 = np.asarray(W1, dtype=np.float32)
    b1 = np.asarray(b1, dtype=np.float32)
    ln_g = np.asarray(ln_g, dtype=np.float32)
    ln_b = np.asarray(ln_b, dtype=np.float32)
    W2 = np.asarray(W2, dtype=np.float32)
    b2 = np.asarray(b2, dtype=np.float32)
    W3 = np.asarray(W3, dtype=np.float32)
    b3 = np.asarray(b3, dtype=np.float32)

    if not (np.allclose(ln_g, 1.0) and np.allclose(ln_b, 0.0)):
        return _np_reference(slots, W1, b1, ln_g, ln_b, W2, b2, W3, b3)

    Wa, Wb, Wc, Wd = W1[:D], W1[D : 2 * D], W1[2 * D : 3 * D], W1[3 * D :]
    WA = Wa + Wc  # [64, 256]
    wbwd = np.concatenate([Wb - Wc, Wd], axis=0)  # [128, 256]
    b3f = float(b3.reshape(-1)[0])

    key = b3f
    if key not in _prog_cache:
        _prog_cache[key] = _build_program(b3f)
    nc = _prog_cache[key]

    bf = ml_dtypes.bfloat16
    wbwd_b = wbwd.astype(bf)
    w2s = np.ascontiguousarray(
        np.transpose(W2.reshape(2, 128, K2), (1, 0, 2))
    ).astype(bf)  # [128h', 2hc, 128k]
    w3s = W3.reshape(K2, 1).astype(bf)
    b2s = b2.reshape(K2, 1).astype(np.float32)

    in_maps = []
    for b in range(B):
        sT = np.ascontiguousarray(slots[b].T)  # [64, 256] f32
        utab_s = (slots[b] @ WA + b1).astype(bf)  # [256, 256]
        in_maps.append(
            {
                "slotst_f": sT,
                "slotst_b": sT.astype(bf),
                "wbwd": wbwd_b,
                "utab": utab_s,
                "w2": w2s,
                "w3": w3s,
                "b2": b2s,
            }
        )

    trace = os.environ.get("KERNEL_TRACE", "0") == "1"
    try:
        res = run_bass_kernel_spmd(nc, in_maps, list(range(NCORES)), trace=trace)
    except ModuleNotFoundError:
        res = run_bass_kernel_spmd(nc, in_maps, list(range(NCORES)), trace=False)
    kernel.last_result = res
    if trace and res.exec_time_ns is not None:
        print(f"HW exec time: {res.exec_time_ns} ns")
        kernel.last_exec_time_ns = res.exec_time_ns
    out = np.stack(
        [res.results[b]["outT"].reshape(N, N).T for b in range(B)], axis=0
    )
    return np.ascontiguousarray(out).astype(np.float32)


kernel.last_exec_time_ns = None
kernel.last_result = None


# revision 28
# speedup vs baseline: 1.1187x; 1.0201x over previous
"""Trainium2 Bass kernel for nn_CausalGraphLearner.

Computes, for each batch b and slot pair (i, j):
    x    = cat([s_i, s_j, s_i - s_j, s_i * s_j])            # [4D]
    h1   = x @ W1 + b1                                      # [H]
    h    = gelu(LayerNorm(h1))                              # exact gelu
    h2   = gelu(h @ W2 + b2)
    out  = sigmoid(h2 @ W3 + b3)                            # scalar
Output: [B, N, N] with B=8, N=256, D=64, H=256.

Strategy: data-parallel over B across the 8 NeuronCores (1 batch per core).
The first Linear factors as
    h1 = s_j@(Wb-Wc) + (s_i*s_j)@Wd + [s_i@(Wa+Wc) + b1]
so per row-index i we run one K=128 matmul (lhsT = [slotsT; s_i*slotsT]) plus
a rank-1 accumulate for the i-dependent row broadcast.  The final Linear
(h2 @ W3) runs as one-column matmuls with z2g as the stationary operand,
producing the output transposed; the host transposes it back.
"""

import os
import sys

sys.path.insert(0, "/opt/trn_rl_repo")

import numpy as np
import ml_dtypes

import concourse.bass as bass
import concourse.tile as tile
from concourse import bacc, mybir
from concourse.bass_utils import run_bass_kernel_spmd

B, N, D = 8, 256, 64
H = 256
K2 = H // 2  # 128
LN_EPS = 1e-5
NCORES = 8

F32 = mybir.dt.float32
BF16 = mybir.dt.bfloat16
U32 = mybir.dt.uint32
I32 = mybir.dt.int32
AF = mybir.ActivationFunctionType
ALU = mybir.AluOpType

MAGIC = 0x5F3759DF  # fast inverse-sqrt seed

_prog_cache = {}


def _build_program(b3: float) -> bass.Bass:
    nc = bacc.Bacc(
        "TRN2", target_bir_lowering=False, debug=False, num_devices=NCORES
    )

    slotst_f = nc.declare_dram_parameter("slotst_f", [D, N], F32, False)
    slotst_b = nc.declare_dram_parameter("slotst_b", [D, N], BF16, False)
    wbwd_d = nc.declare_dram_parameter("wbwd", [2 * D, H], BF16, False)
    utab_d = nc.declare_dram_parameter("utab", [N, H], BF16, False)
    w2_d = nc.declare_dram_parameter("w2", [128, 2, K2], BF16, False)
    w3_d = nc.declare_dram_parameter("w3", [K2, 1], BF16, False)
    b2_d = nc.declare_dram_parameter("b2", [K2, 1], F32, False)
    # outT[j, i] = sigmoid-logit for pair (i, j); host transposes back.
    outT_d = nc.declare_dram_parameter("outT", [2, 128, 2, 128], F32, True)
    acts_d = nc.dram_tensor("actscratch", [3, 8, N, H], BF16)

    NH = 5   # h1 psum ring depth (banks)
    BATCH = 4  # stats-merge batch (i's per merge)

    with tile.TileContext(nc) as tc:
        with (
            tc.tile_pool(name="const", bufs=1) as cpool,
            tc.tile_pool(name="work", bufs=1) as wpool,
            tc.tile_pool(name="tmp", bufs=2) as spool,
            tc.tile_pool(name="psum", bufs=1, space="PSUM") as ppool,
        ):
            # ---- constants / parameters in SBUF ----
            combs = [cpool.tile([128, N], BF16, name=f"comb{k}", tag=f"comb{k}") for k in range(8)]
            slotshi = cpool.tile([128, N], F32, name="slotshi", tag="slotshi")
            slotsbh = cpool.tile([128, N], BF16, name="slotsbh", tag="slotsbh")
            wbwd = cpool.tile([128, H], BF16, name="wbwd", tag="wbwd")
            ustage = [cpool.tile([1, BATCH, H], BF16, name=f"ustage{k}", tag=f"ustage{k}") for k in range(3)]
            w2t = cpool.tile([128, 2, K2], BF16, name="w2", tag="w2")
            w3t = cpool.tile([K2, 1], BF16, name="w3", tag="w3")
            b2t = cpool.tile([K2, 1], F32, name="b2", tag="b2")
            ones = cpool.tile([1, 128], BF16, name="ones", tag="ones")
            b3t = cpool.tile([128, 1], F32, name="b3t", tag="b3t")

            for k in range(8):
                nc.sync.dma_start(combs[k][0:D, :], slotst_b[:, :])
            nc.sync.dma_start(slotshi[D:128, :], slotst_f[:, :])
            nc.sync.dma_start(slotsbh[D:128, :], slotst_b[:, :])
            nc.sync.dma_start(wbwd[:], wbwd_d[:, :])
            nc.sync.dma_start(w2t[:], w2_d[:, :, :])
            nc.sync.dma_start(w3t[:], w3_d[:, :])
            nc.sync.dma_start(b2t[:], b2_d[:, :])
            nc.vector.memset(ones[:], 1.0)
            nc.vector.memset(b3t[:], float(b3) * 0.5)

            # ---- PSUM layout: 5 + 2 + 1 = 8 banks exactly ----
            h1r = [ppool.tile([128, 2, H], F32, name=f"h1_{m}", tag=f"h1_{m}") for m in range(NH)]
            z2p = [ppool.tile([128, 2, N], F32, name=f"z2p{m}", tag=f"z2p{m}") for m in range(2)]
            lT = ppool.tile([128, 2, 2, 128], F32, name="lT", tag="lT")

            # ---- SBUF work rings ----
            actr = [wpool.tile([128, BATCH, 2, H], BF16, name=f"act{m}", tag=f"act{m}") for m in range(3)]
            actT8 = [wpool.tile([128, 2, 8, N], BF16, name=f"actT8_{m}", tag=f"actT8_{m}") for m in range(2)]
            z2g = [wpool.tile([128, 2, N], BF16, name=f"z2g{m}", tag=f"z2g{m}") for m in range(4)]
            stats = [wpool.tile([128, BATCH, 2, 6], F32, name=f"stats{m}", tag=f"stats{m}") for m in range(3)]
            mvar = [wpool.tile([128, BATCH, 2, 2], F32, name=f"mvar{m}", tag=f"mvar{m}") for m in range(3)]
            rstd = [wpool.tile([128, BATCH, 2], F32, name=f"rstd{m}", tag=f"rstd{m}") for m in range(3)]
            nbias = [wpool.tile([128, BATCH, 2], F32, name=f"nbias{m}", tag=f"nbias{m}") for m in range(3)]
            sig = [wpool.tile([128, 2, 128], F32, name=f"sig{m}", tag=f"sig{m}") for m in range(2)]
            outsb = [wpool.tile([128, 2, 128], F32, name=f"outsb{m}", tag=f"outsb{m}") for m in range(2)]

            def merge_and_rsqrt(k: int):
                """bn_aggr folds each (i, c) window pair into (mean, var) in
                one op; then a short Newton-rsqrt chain on the batch."""
                w = k % 3
                for bi in range(BATCH):
                    for c in range(2):
                        nc.vector.bn_aggr(
                            mvar[w][:, bi, c, :], stats[w][:, bi, c, :]
                        )
                shp = [128, BATCH, 2]
                mean = mvar[w][:, :, :, 0]
                var = mvar[w][:, :, :, 1]

                tvar = spool.tile(shp, F32, tag="tvar")
                nc.vector.tensor_scalar(tvar[:], var, 1.0, LN_EPS, ALU.mult, ALU.add)
                # Newton rsqrt with bit-trick seed: r0_bits = MAGIC - (bits>>1)
                ti = spool.tile(shp, I32, tag="ti")
                nc.vector.tensor_scalar(
                    ti[:], tvar[:].bitcast(I32), 1, None, ALU.logical_shift_right
                )
                nc.vector.tensor_scalar(ti[:], ti[:], -1, MAGIC, ALU.mult, ALU.add)
                r = ti[:].bitcast(F32)
                ta = spool.tile(shp, F32, tag="ta")
                tb2 = spool.tile(shp, F32, tag="tb2")
                dest = rstd[w]
                nc.vector.tensor_tensor(ta[:], r, r, ALU.mult)
                nc.vector.tensor_tensor(ta[:], ta[:], tvar[:], ALU.mult)
                nc.vector.tensor_scalar(tb2[:], ta[:], -0.5, 1.5, ALU.mult, ALU.add)
                nc.vector.tensor_tensor(dest[:], r, tb2[:], ALU.mult)
                # nbias = -mean * rstd
                tA = spool.tile(shp, F32, tag="tA")
                nc.vector.tensor_tensor(tA[:], mean, dest[:], ALU.mult)
                nc.vector.tensor_scalar(nbias[w][:], tA[:], -1.0, None, ALU.mult)

            # ---- main loop, software-pipelined in batches of BATCH ----
            NB = N // BATCH

            def stage_u(k: int):
                """Prefetch batch k's u-rows into the ustage ring."""
                if k >= NB:
                    return
                nc.gpsimd.dma_start(
                    ustage[k % 3][0:1, :, :],
                    utab_d[BATCH * k : BATCH * (k + 1), :].rearrange(
                        "(o a) b -> o a b", o=1
                    ),
                )

            def phase_mt(k: int):
                """mT multiplies for batch k (emitted one batch early so the
                DVE queue never blocks the next batch's matmuls)."""
                if k >= NB:
                    return
                for i in range(BATCH * k, BATCH * (k + 1)):
                    mc = i % 8
                    # mT = s_i * slotsT on partitions 64..127 (bf16 in, DVE)
                    nc.vector.tensor_scalar(
                        combs[mc][D:128, :],
                        slotsbh[D:128, :],
                        slotshi[D:128, i : i + 1],
                        None,
                        ALU.mult,
                    )

            def phase_a(k: int):
                """mm1 + bn_stats for the 4 i's of batch k."""
                stage_u(k + 2)
                phase_mt(k + 1)
                for i in range(BATCH * k, BATCH * (k + 1)):
                    m5 = i % NH
                    mc = i % 8
                    bi = i % BATCH

                    # h1 = comb.T @ [WB; Wd]  (+ rank-1 of (u_i + b1))
                    h1 = h1r[m5]
                    nc.tensor.matmul(
                        h1[:, 0, :], combs[mc][:, 0:128], wbwd[:], start=True, stop=False
                    )
                    nc.tensor.matmul(
                        h1[:, 1, :], combs[mc][:, 128:256], wbwd[:], start=False, stop=False
                    )
                    urow = ustage[(i // BATCH) % 3][0:1, bi, :]
                    nc.tensor.matmul(h1[:, 0, :], ones[:], urow, start=False, stop=False)
                    nc.tensor.matmul(h1[:, 1, :], ones[:], urow, start=False, stop=True)
                for i in range(BATCH * k, BATCH * (k + 1)):
                    m5 = i % NH
                    w = k % 3
                    bi = i % BATCH
                    h1 = h1r[m5]
                    # LayerNorm stats (per j-chunk; grouped bn_stats would be
                    # flattened by AP opt and mix the chunks)
                    for c in range(2):
                        nc.vector.bn_stats(stats[w][:, bi, c, :], h1[:, c, :])

            def phase_b(k: int):
                """gelu1 + act scratch write for the 4 i's of batch k."""
                m3a = k % 3
                for i in range(BATCH * k, BATCH * (k + 1)):
                    m5 = i % NH
                    w = k % 3
                    bi = i % BATCH
                    h1 = h1r[m5]

                    # act = gelu((h1 - mean) * rstd)  [per-chunk scale/bias]
                    for c in range(2):
                        nc.scalar.activation(
                            actr[m3a][:, bi, c, :],
                            h1[:, c, :],
                            AF.Gelu,
                            bias=nbias[w][:, bi, c : c + 1],
                            scale=rstd[w][:, bi, c : c + 1],
                        )
                # stage the whole batch's act to DRAM scratch (SWDGE, one DMA)
                nc.gpsimd.dma_start(
                    acts_d[(k // 2) % 3, BATCH * (k % 2) : BATCH * (k % 2 + 1), :, :]
                    .rearrange("i (c p) h -> p i c h", c=2),
                    actr[m3a][:, :, :, :],
                )

            def phase_c_dma(m: int):
                """batched DMA transposes for block m (emitted right after the
                block's last acts write so the transfer overlaps compute)."""
                m2 = m % 2
                # actT[h', d, ip, j] = act[i=8m+ip][j, 128d + h']
                for d in range(2):
                    nc.sync.dma_start_transpose(
                        actT8[m2][:, d, :, :].rearrange("p a b -> p (a b)"),
                        acts_d[m % 3, :, :, 128 * d : 128 * (d + 1)].rearrange(
                            "a b c -> (a b) c"
                        ),
                    )

            def phase_c(m: int):
                """block of 8 i's: mm2/gelu2/mm3 on pre-transposed acts."""
                m2 = m % 2
                for i in range(8 * m, 8 * (m + 1)):
                    ip = i % 8
                    blk = i // 128
                    pair = i // 2

                    # z2T[k, j] = W2.T-chunks @ actT  (double-buffered per pair)
                    zp = z2p[pair % 2]
                    for hc in range(2):
                        nc.tensor.matmul(
                            zp[:, i % 2, :],
                            w2t[:, hc, :],
                            actT8[m2][:, hc, ip, :],
                            start=(i % 2 == 0 and hc == 0),
                            stop=(i % 2 == 1 and hc == 1),
                        )

                    if i % 2 == 1:
                        # gelu2 batched over the pair; b2 is a per-partition bias
                        nc.scalar.activation(
                            z2g[pair % 4][:, :, :],
                            zp[:, :, :],
                            AF.Gelu,
                            bias=b2t[:, 0:1],
                            scale=1.0,
                        )
                        # logits for the pair: one-column matmuls with z2g as
                        # the stationary operand; row lands transposed in lT
                        for par in range(2):
                            ii = i - 1 + par
                            it = ii % 128
                            for c in range(2):
                                nc.tensor.matmul(
                                    lT[:, blk % 2, c, it : it + 1],
                                    z2g[pair % 4][:, par, 128 * c : 128 * (c + 1)],
                                    w3t[:],
                                    start=True,
                                    stop=True,
                                )

                    if i % 128 == 127:
                        # sigmoid(x + b3) = 0.5 + 0.5*tanh((x + b3)/2); tanh is
                        # in the gelu table set, so no ACT table reload.
                        nc.scalar.activation(
                            sig[blk % 2][:],
                            lT[:, blk % 2, :, :],
                            AF.Tanh,
                            bias=b3t[:, 0:1],
                            scale=0.5,
                        )
                        nc.vector.tensor_scalar(
                            outsb[blk % 2][:], sig[blk % 2][:], 0.5, 0.5, ALU.mult, ALU.add
                        )
                        nc.gpsimd.dma_start(
                            outT_d[:, :, blk, :].rearrange("c p t -> p c t"),
                            outsb[blk % 2][:],
                        )

            stage_u(0)
            stage_u(1)
            phase_mt(0)
            for k in range(NB):
                phase_a(k)
                merge_and_rsqrt(k)
                # phase_c compute for a block whose acts are 2+ batches old
                # (transposes were issued at the end of cycle 2m+1)
                if k % 2 == 0 and k >= 4:
                    phase_c(k // 2 - 2)
                phase_b(k)
                if k % 2 == 1:
                    phase_c_dma((k - 1) // 2)
            phase_c(NB // 2 - 2)
            phase_c(NB // 2 - 1)

    nc.finalize()
    return nc


def _np_reference(slots, W1, b1, ln_g, ln_b, W2, b2, W3, b3):
    """Exact fallback (only used if ln_g/ln_b are not identity)."""
    import jax
    import jax.numpy as jnp

    si = slots[:, :, None, :]
    sj = slots[:, None, :, :]
    d = slots.shape[-1]
    Wa, Wb, Wc, Wd = W1[:d], W1[d : 2 * d], W1[2 * d : 3 * d], W1[3 * d :]
    h = (
        jnp.einsum("bnd,dh->bnh", slots, Wa + Wc)[:, :, None, :]
        + jnp.einsum("bnd,dh->bnh", slots, Wb - Wc)[:, None, :, :]
        + jnp.einsum("bxyd,dh->bxyh", si * sj, Wd)
        + b1
    )
    mu = jnp.mean(h, axis=-1, keepdims=True)
    var = jnp.mean(jnp.square(h - mu), axis=-1, keepdims=True)
    h = (h - mu) * jax.lax.rsqrt(var + LN_EPS) * ln_g + ln_b
    h = jax.nn.gelu(h, approximate=False)
    h = jax.nn.gelu(jnp.einsum("bxyh,hk->bxyk", h, W2) + b2, approximate=False)
    logits = (jnp.einsum("bxyk,ko->bxyo", h, W3) + b3)[..., 0]
    return np.asarray(jax.nn.sigmoid(logits), dtype=np.float32)


def kernel(slots, W1, b1, ln_g, ln_b, W2, b2, W3, b3):
    slots = np.asarray(slots, dtype=np.float32)
    W1 = np.asarray(W1, dtype=np.float32)
    b1 = np.asarray(b1, dtype=np.float32)
    ln_g = np.asarray(ln_g, dtype=np.float32)
    ln_b = np.asarray(ln_b, dtype=np.float32)
    W2 = np.asarray(W2, dtype=np.float32)
    b2 = np.asarray(b2, dtype=np.float32)
    W3 = np.asarray(W3, dtype=np.float32)
    b3 = np.asarray(b3, dtype=np.float32)

    if not (np.allclose(ln_g, 1.0) and np.allclose(ln_b, 0.0)):
        return _np_reference(slots, W1, b1, ln_g, ln_b, W2, b2, W3, b3)

    Wa, Wb, Wc, Wd = W1[:D], W1[D : 2 * D], W1[2 * D : 3 * D], W1[3 * D :]
    WA = Wa + Wc  # [64, 256]
    wbwd = np.concatenate([Wb - Wc, Wd], axis=0)  # [128, 256]
    b3f = float(b3.reshape(-1)[0])

    key = b3f
    if key not in _prog_cache:
        _prog_cache[key] = _build_program(b3f)
    nc = _prog_cache[key]

    bf = ml_dtypes.bfloat16
    wbwd_b = wbwd.astype(bf)
    w2s = np.ascontiguousarray(
        np.transpose(W2.reshape(2, 128, K2), (1, 0, 2))
    ).astype(bf)  # [128h', 2hc, 128k]
    w3s = W3.reshape(K2, 1).astype(bf)
    b2s = b2.reshape(K2, 1).astype(np.float32)

    in_maps = []
    for b in range(B):
        sT = np.ascontiguousarray(slots[b].T)  # [64, 256] f32
        utab_s = (slots[b] @ WA + b1).astype(bf)  # [256, 256]
        in_maps.append(
            {
                "slotst_f": sT,
                "slotst_b": sT.astype(bf),
                "wbwd": wbwd_b,
                "utab": utab_s,
                "w2": w2s,
                "w3": w3s,
                "b2": b2s,
            }
        )

    trace = os.environ.get("KERNEL_TRACE", "0") == "1"
    try:
        res = run_bass_kernel_spmd(nc, in_maps, list(range(NCORES)), trace=trace)
    except ModuleNotFoundError:
        res = run_bass_kernel_spmd(nc, in_maps, list(range(NCORES)), trace=False)
    kernel.last_result = res
    if trace and res.exec_time_ns is not None:
        print(f"HW exec time: {res.exec_time_ns} ns")
        kernel.last_exec_time_ns = res.exec_time_ns
    out = np.stack(
        [res.results[b]["outT"].reshape(N, N).T for b in range(B)], axis=0
    )
    return np.ascontiguousarray(out).astype(np.float32)


kernel.last_exec_time_ns = None
kernel.last_result = None
